# revision 17
# baseline (speedup 1.0000x reference)
"""Multi-head self-attention on 8 Trainium2 NeuronCores.

Problem: B=4, S=2048, D=1024, H=16 heads (dk=64), torch-Linear style
projections (y = x @ W.T + b), softmax attention, output projection.

Sharding: 8 cores = 4 batches x 2 head-groups (8 heads each).  Each core
computes, for its (batch b, group g):
    QT = (Wq_g/(4*sqrt(dk))) @ x_b.T + bq_g/(4*sqrt(dk))  [512, S]
         (scores are produced pre-divided by 4; the exp stage multiplies
          back: ACT uses Exp scale=4, DVE evaluates p(u)^4 with p ~ e^u)
    KT = Wk_g @ x_b.T            [512, S]  (bk dropped: it shifts scores
                                  uniformly per query, cancels in softmax)
    V  = x_b @ Wv_g.T            [S, 512]  (keys on partitions; bv is folded
                                  into the host bias as Wo @ bv since the
                                  softmax weights sum to one)
    per head pair (h0, h1): scoresT = K_h @ Q_h.T  [S(keys), S(queries)]
      - the two heads' score matmuls are 64-contraction row-tiled and run
        CONCURRENTLY in the PE array (rows 0:63 / 64:127)
    expT = exp(scoresT)          (no max-subtraction: |scores| < ~3.5)
        - scalar engine (ACT) exponentiates head h0's scores (Exp, scale=4)
        - vector engine (DVE) head h1's, via one custom micro-coded op
          p(u)^4, p = deg-3 rel-minimax of e^u
    PV via COL-TILED concurrent pairs: stationary V_h0 (64 cols) at
      tile_position (0,0) and V_h1 at (0,64) write one PSUM bank holding
      [outT_h0 ; outT_h1] row-aligned; an identical ones[128,64] stationary
      pair writes a second bank with [den_h0 ; den_h1].  One DVE divide
      (pv / den) then normalizes BOTH heads at once -- no cross-partition
      swap, no reciprocal+mult chain.
    partialT = Wo_g @ onorm_all  [1024, S] in bf16
Host sums the two group partials per batch, transposes, adds bo + Wo@bv.

PE mode switches (row-tiled scores <-> col-tiled PV/DEN) drain the array
(~100ns); ticks are emitted in BATCHES OF TWO so each batch pays only two
mode transitions: [S(t) S(t+1)] [PV/DEN(t-3) PV/DEN(t-2)].

Device dtypes: bf16 matmul operands, f32 PSUM/exp/normalization, bf16 out.
"""

import math

import numpy as np
import ml_dtypes

import concourse.bass as bass
import concourse.bacc as bacc_mod
import concourse.mybir as mybir
import concourse.tile as tile
from concourse.bass_utils import run_bass_kernel_spmd

BF16 = mybir.dt.bfloat16
F32 = mybir.dt.float32
F8E4 = mybir.dt.float8e4
AF = mybir.ActivationFunctionType
DR = mybir.MatmulPerfMode.DoubleRow

B, S, D, H = 4, 2048, 1024, 16
DK = D // H  # 64
NCORES = 8
GROUPS = 2  # tensor-parallel head groups
DG = D // GROUPS  # 512 features per group
P = 128
FT = DG // P  # 4 feature tiles per group == head pairs

# exp: scores are pre-scaled by 1/SC; ACT uses Exp(scale=SC) on head h0,
# DVE evaluates p(u)^4 on head h1 in ONE custom op with
# p = c0 + u*(c1 + u*(c2 + u*c3)) ~ e^u (c3 rides in via the Src1 spill).
SC = 4.0
# fp8 path: Q/K projections run as e4m3 DoubleRow matmuls (2 contraction
# rows per PE cell, ~1.8x).  Weights are pre-boosted by WBOOST so their
# entries sit in e4m3's normal range; the boost is divided back out in the
# PSUM->SBUF activation copy (scale=).  V / Wo stay bf16: their quantization
# noise would hit the output directly, while q/k noise only perturbs scores.
WBOOST = 64.0
EXP_C0 = 0.99773437
EXP_C1 = 1.0064234
EXP_C2 = 0.5302388
EXP_C3 = 0.16039258

# fused divide out = in0/in1 via the BITWISE_NOT exponent-flip seed:
# 1/b = ~b · g(t) with t = b·~b ∈ [-4.5, -4]; g = deg-2 rel-minimax of 1/t
# (max rel err 5.1e-5).  out = (in0·~b)·(c0 + t·(c1 + t·c2)) — 8 ALU ops,
# exactly the DVE slice budget, one instruction.
DIV_C0 = -0.7071066070499812
DIV_C1 = -0.16652203635848312
DIV_C2 = -0.013060548941608689

_DVE_OPS = {}
DEBUG_DUMP = False


def _register_dve_exp_ops():
    """Register the custom DVE micro-ops (exp path + fused divide).
    Idempotent; appends to concourse's module-level op registry."""
    if _DVE_OPS:
        return _DVE_OPS
    from concourse.dve_spec import (
        Spec, Src0, Src1, C0, C1, C2, C3, Bin, AluOp, sq, lower, _has_src1,
        _spill_c3_to_src1,
    )
    from concourse.dve_uop import DveOpSpec
    from concourse.dve_ops import (
        DveOp,
        OPS,
        CUSTOM_DVE_SPECS,
        _SUB_OPCODE_FOR_NAME,
        _CUSTOM_DVE_ROW_BASE,
    )

    u = Src0

    def _ref_exp4p(in0, in1, s0, s1, imm2):
        p = s0 + in0 * (s1 + in0 * (imm2 + in0 * in1))
        return (p * p) ** 2

    _notb = Bin(AluOp.BITWISE_NOT, Src1, Src1)
    _t = Src1 * _notb

    def _ref_divab(in0, in1, s0, s1, imm2):
        import numpy as np

        notb = (~in1.view(np.int32)).view(np.float32)
        t = in1 * notb
        return (in0 * notb) * (s0 + t * (s1 + t * imm2))

    specs = {
        "ANT_EXP4P": Spec(
            body=_spill_c3_to_src1(sq(sq(C0 + u * (C1 + u * (C2 + u * C3))))),
            reference=_ref_exp4p,
        ),
        "ANT_DIVAB": Spec(
            body=(Src0 * _notb) * (C0 + _t * (C1 + _t * C2)),
            reference=_ref_divab,
        ),
    }
    for name, sp in specs.items():
        if name not in _SUB_OPCODE_FOR_NAME:
            row = _CUSTOM_DVE_ROW_BASE + len(OPS)
            op = DveOp(name, sp, subdim=False, uops_sha={})
            _SUB_OPCODE_FOR_NAME[name] = row
            OPS.append(op)
            CUSTOM_DVE_SPECS[name] = sp
            for ver in ("v3", "v4"):
                s = DveOpSpec(
                    name=name, opcode=row, uops=lower(sp, ver=ver),
                    rd1_en=_has_src1(sp),
                )
                op.uops_sha[ver] = s.sha(ver)
        else:
            op = next(o for o in OPS if o.name == name)
        _DVE_OPS[name] = op
    return _DVE_OPS


def build_attention_nc(seq: int = S) -> bass.Bass:
    KB = seq // P  # key blocks
    DKB = D // P  # 8 contraction blocks for projections
    QH = min(512, seq)  # query stripe processed per attention pass
    NQH = seq // QH
    QC = min(512, QH)  # matmul moving-operand chunk
    NQC = seq // QC  # chunks per full seq
    DT = D // P
    XC = 512  # x DMA column-chunk width
    NXC = seq // XC

    ops = _register_dve_exp_ops()
    exp4p = ops["ANT_EXP4P"]
    divab = ops["ANT_DIVAB"]

    nc = bacc_mod.Bacc("TRN2", num_devices=NCORES)
    xt_d = nc.declare_dram_parameter("xt", [D, seq], BF16, isOutput=False)
    xt8_d = nc.declare_dram_parameter("xt8", [D, seq], F8E4, isOutput=False)
    wq8_d = nc.declare_dram_parameter("wq8", [D, DG], F8E4, isOutput=False)
    wk8_d = nc.declare_dram_parameter("wk8", [D, DG], F8E4, isOutput=False)
    wvt_d = nc.declare_dram_parameter("wvt", [D, DG], BF16, isOutput=False)
    wot_d = nc.declare_dram_parameter("wot", [DG, D], BF16, isOutput=False)
    bq_d = nc.declare_dram_parameter("bqs", [P, FT], F32, isOutput=False)
    out_d = nc.declare_dram_parameter("out", [D, seq], BF16, isOutput=True)
    dbg_d = None
    if DEBUG_DUMP:
        dbg_d = {
            name: nc.declare_dram_parameter(f"dbg_{name}", [P, FT * seq], BF16,
                                            isOutput=True)
            for name in ("qt", "kt", "onorm")
        }
        dbg_d["v"] = nc.declare_dram_parameter("dbg_v", [P, seq // P * DG],
                                               BF16, isOutput=True)

    with tile.TileContext(nc) as tc:
        with tc.tile_pool(name="persist", bufs=1) as persist:
            bq_sb = persist.tile([P, FT], F32, name="bq_sb")
            nc.sync.dma_start(bq_sb, bq_d[:, :])
            c3_sb = persist.tile([P, 1], F32, name="c3_sb")
            nc.vector.memset(c3_sb, EXP_C3)

            qt_sb = [persist.tile([P, seq], BF16, name=f"qt{i}") for i in range(FT)]
            kt_sb = [persist.tile([P, seq], BF16, name=f"kt{i}") for i in range(FT)]
            # plain V tiles: [128 keys, 512 features] per key block
            v_sb = [persist.tile([P, DG], BF16, name=f"v{i}") for i in range(KB)]
            ones_sb = persist.tile([P, DK], BF16, name="ones_sb")
            nc.vector.memset(ones_sb, 1.0)
            wot_sb = [persist.tile([P, D], BF16, name=f"wot{i}") for i in range(FT)]
            onorm = [persist.tile([P, seq], BF16, name=f"onorm{i}") for i in range(FT)]

            # ---------------- phase 1: projections ----------------
            with (
                tc.tile_pool(name="xw", bufs=1) as xw_pool,
                tc.tile_pool(name="pps", bufs=4, space="PSUM") as proj_ps,
            ):
                # Slab tiles: each holds all 8 contraction sub-tiles side by
                # side so ONE dma_start loads a whole slab.
                xt_all = xw_pool.tile([P, DKB * seq], BF16, name="xts")
                x8_all = xw_pool.tile([P, DKB * seq], F8E4, name="x8s")
                wq_all = xw_pool.tile([P, DKB * DG], F8E4, name="wq8s")
                wk_all = xw_pool.tile([P, DKB * DG], F8E4, name="wk8s")
                wv_all = xw_pool.tile([P, DKB * DG], BF16, name="wvts")
                xt_sb = [xt_all[:, i * seq : (i + 1) * seq] for i in range(DKB)]
                wvt_sb = [wv_all[:, i * DG : (i + 1) * DG] for i in range(DKB)]
                # [partition, k-block, col] views for the DoubleRow APs
                x8_v = x8_all.rearrange("p (k s) -> p k s", s=seq)
                wq_v = wq_all.rearrange("p (k g) -> p k g", g=DG)
                wk_v = wk_all.rearrange("p (k g) -> p k g", g=DG)

                def wdma(dst_all, w_d):
                    return (
                        dst_all.rearrange("p (k g) -> p k g", g=DG),
                        w_d.rearrange("(k p) g -> p k g", p=P),
                    )

                def xsrc(c, dst_all=None, src_d=None):
                    csl = slice(c * XC, (c + 1) * XC)
                    return (
                        dst_all.rearrange("p (k s) -> p k s", s=seq)[:, :, csl],
                        src_d.rearrange("(k p) s -> p k s", p=P)[:, :, csl],
                    )

                # fp8 x first (gates QT), then the small fp8 weights, then
                # the bf16 x (V) and remaining weights.
                x8d0, x8s0 = xsrc(0, x8_all, xt8_d)
                nc.sync.dma_start(x8d0[:, 0:4, :], x8s0[:, 0:4, :])
                nc.sync.dma_start(x8d0[:, 4:8, :], x8s0[:, 4:8, :])
                wd, ws = wdma(wq_all, wq8_d)
                nc.scalar.dma_start(wd[:, 0:4, :], ws[:, 0:4, :])
                nc.scalar.dma_start(wd[:, 4:8, :], ws[:, 4:8, :])
                nc.scalar.dma_start(*wdma(wk_all, wk8_d))
                for c in range(1, NXC):
                    (nc.sync if c % 2 else nc.scalar).dma_start(
                        *xsrc(c, x8_all, xt8_d))
                for c in range(NXC):
                    (nc.scalar if c % 2 else nc.sync).dma_start(
                        *xsrc(c, xt_all, xt_d))
                nc.scalar.dma_start(*wdma(wv_all, wvt_d))
                for ft in range(FT):
                    nc.sync.dma_start(wot_sb[ft], wot_d[ft * P : (ft + 1) * P, :])

                # QT / KT: features on partitions, queries on free dim.
                # e4m3 DoubleRow: each matmul consumes TWO 128-row
                # contraction blocks ([p, 2, cols] APs).
                QSC = 1.0 / (WBOOST * SC * math.sqrt(DK))
                KSC = 1.0 / WBOOST
                for c in range(NQC):
                    csl = slice(c * QC, (c + 1) * QC)
                    for ft in range(FT):
                        fsl = slice(ft * P, (ft + 1) * P)
                        psq = proj_ps.tile([P, QC], F32, name="psq", tag="proj")
                        for k2 in range(DKB // 2):
                            nc.tensor.matmul(
                                psq,
                                lhsT=wq_v[:, 2 * k2 : 2 * k2 + 2, fsl],
                                rhs=x8_v[:, 2 * k2 : 2 * k2 + 2, csl],
                                start=k2 == 0,
                                stop=k2 == DKB // 2 - 1,
                                perf_mode=DR,
                            )
                        nc.scalar.activation(
                            qt_sb[ft][:, csl], psq, AF.Identity,
                            bias=bq_sb[:, ft : ft + 1], scale=QSC,
                        )
                    for ft in range(FT):
                        fsl = slice(ft * P, (ft + 1) * P)
                        psk = proj_ps.tile([P, QC], F32, name="psk", tag="proj")
                        for k2 in range(DKB // 2):
                            nc.tensor.matmul(
                                psk,
                                lhsT=wk_v[:, 2 * k2 : 2 * k2 + 2, fsl],
                                rhs=x8_v[:, 2 * k2 : 2 * k2 + 2, csl],
                                start=k2 == 0,
                                stop=k2 == DKB // 2 - 1,
                                perf_mode=DR,
                            )
                        nc.scalar.activation(
                            kt_sb[ft][:, csl], psk, AF.Identity, scale=KSC,
                        )

                # V: keys on partitions, features on free dim.  bv is folded
                # into the host-side bias.  Plain contiguous copy (scalar
                # engine; idle here).
                for kb in range(KB):
                    ksl = slice(kb * P, (kb + 1) * P)
                    psv = proj_ps.tile([P, DG], F32, name="psv", tag="proj")
                    for k in range(DKB):
                        nc.tensor.matmul(
                            psv,
                            lhsT=xt_sb[k][:, ksl],
                            rhs=wvt_sb[k],
                            start=k == 0,
                            stop=k == DKB - 1,
                        )
                    nc.scalar.activation(v_sb[kb], psv, AF.Copy)

            # ---------------- phase 2: attention ----------------
            with (
                tc.tile_pool(name="sps", bufs=2, space="PSUM") as s_ps,
                tc.tile_pool(name="pvps", bufs=2, space="PSUM") as pv_ps,
                tc.tile_pool(name="epool", bufs=6) as e_pool,
                tc.tile_pool(name="dpool", bufs=2) as den_pool,
                tc.tile_pool(name="osb", bufs=4) as o_sb_pool,
            ):
                # Ticks t = (qh, pr, kb) linearized; emitted in batches of
                # two so the PE pays one row->col and one col->row mode
                # drain per TWO ticks.  PV/DEN for tick t are emitted at
                # batch containing t+DEPTH.
                DEPTH = 4  # must be EVEN so (t-DEPTH, t-DEPTH+1) aligns
                # with tick-pair boundaries (kb0's start=True first)
                TICKS = NQH * FT * KB

                def tick_coords(t):
                    qh, r = divmod(t, FT * KB)
                    pr, kb = divmod(r, KB)
                    return qh, pr, kb

                epipe = {}  # t -> (pv, den, pr, kb, e0, e1)
                pend = []  # (due_batch, closure) for norm ops
                pvden = {}  # pr-slot -> (pv, den) current accumulators

                def emit_scores(t):
                    qh, pr, kb = tick_coords(t)
                    qsl = slice(qh * QH, (qh + 1) * QH)
                    ksl = slice(kb * P, (kb + 1) * P)
                    s0 = s_ps.tile([P, QH], F32, name="s0", tag="s0")
                    s1 = s_ps.tile([P, QH], F32, name="s1", tag="s1")
                    nc.tensor.matmul(
                        s0,
                        lhsT=kt_sb[pr][0:64, ksl],
                        rhs=qt_sb[pr][0:64, qsl],
                        start=True, stop=True,
                    )
                    nc.tensor.matmul(
                        s1,
                        lhsT=kt_sb[pr][64:128, ksl],
                        rhs=qt_sb[pr][64:128, qsl],
                        start=True, stop=True,
                    )
                    e0 = e_pool.tile([P, QH], BF16, name="e0", tag="e0")
                    e1 = e_pool.tile([P, QH], BF16, name="e1", tag="e1")
                    nc.scalar.activation(e0, s0, AF.Exp, scale=SC)
                    nc.vector._custom_dve(
                        exp4p, out=e1, in0=s1, in1=c3_sb[:, 0:1],
                        s0=EXP_C0, s1=EXP_C1, imm2=EXP_C2,
                    )
                    if kb == 0:
                        pv = pv_ps.tile([P, QH], F32, name="pv", tag="pv")
                        den = pv_ps.tile([P, QH], F32, name="den", tag="den")
                        pvden[t // KB] = (pv, den)
                    pv, den = pvden[t // KB]
                    epipe[t] = (pv, den, pr, kb, e0, e1)

                def emit_pvden(t):
                    pv, den, pr, kb, e0, e1 = epipe.pop(t)
                    vc = slice(pr * P, pr * P + 64)
                    vc1 = slice(pr * P + 64, (pr + 1) * P)
                    nc.tensor.matmul(
                        pv[0:64, :], lhsT=v_sb[kb][:, vc], rhs=e0,
                        start=kb == 0, stop=kb == KB - 1,
                        tile_position=(0, 0),
                    )
                    nc.tensor.matmul(
                        pv[64:128, :], lhsT=v_sb[kb][:, vc1], rhs=e1,
                        start=kb == 0, stop=kb == KB - 1,
                        tile_position=(0, 64),
                    )
                    nc.tensor.matmul(
                        den[0:64, :], lhsT=ones_sb, rhs=e0,
                        start=kb == 0, stop=kb == KB - 1,
                        tile_position=(0, 0),
                    )
                    nc.tensor.matmul(
                        den[64:128, :], lhsT=ones_sb, rhs=e1,
                        start=kb == 0, stop=kb == KB - 1,
                        tile_position=(0, 64),
                    )
                    if kb == KB - 1:
                        qh, pr_, _ = tick_coords(t)
                        qsl = slice(qh * QH, (qh + 1) * QH)

                        # DVE may read only ONE operand from PSUM: the
                        # scalar engine (same act table as Exp, no table
                        # reload) first copies den to SBUF, then ONE custom
                        # DVE fused-divide normalizes both heads at once.
                        state = {}

                        def norm_copy(den=den, state=state):
                            den_sb = den_pool.tile([P, QH], F32,
                                                   name="den_sb", tag="den_sb")
                            nc.scalar.activation(den_sb, den, AF.Copy)
                            state["den_sb"] = den_sb

                        def norm_div(pv=pv, pr_=pr_, qsl=qsl, state=state):
                            nc.vector._custom_dve(
                                divab, out=onorm[pr_][:, qsl], in0=pv,
                                in1=state["den_sb"],
                                s0=DIV_C0, s1=DIV_C1, imm2=DIV_C2,
                            )

                        pend.append((t // 2 + 2, norm_copy))
                        pend.append((t // 2 + 3, norm_div))

                def flush_due(bi):
                    while pend and pend[0][0] <= bi:
                        pend.pop(0)[1]()
                    pend.sort(key=lambda e: e[0])

                for bi in range(TICKS // 2):
                    t0 = 2 * bi
                    emit_scores(t0)
                    emit_scores(t0 + 1)
                    c0 = t0 - DEPTH  # always odd-aligned pair (c0, c0+1)
                    if c0 >= 0:
                        emit_pvden(c0)
                        emit_pvden(c0 + 1)
                    flush_due(bi)
                # drain the pipe
                for t in sorted(epipe.keys()):
                    emit_pvden(t)
                flush_due(1 << 30)
                while pend:
                    pend.pop(0)[1]()

                if DEBUG_DUMP:
                    for i in range(FT):
                        nc.sync.dma_start(
                            dbg_d["qt"][:, i * seq:(i + 1) * seq], qt_sb[i])
                        nc.sync.dma_start(
                            dbg_d["kt"][:, i * seq:(i + 1) * seq], kt_sb[i])
                        nc.sync.dma_start(
                            dbg_d["onorm"][:, i * seq:(i + 1) * seq], onorm[i])
                    for kb in range(KB):
                        nc.sync.dma_start(
                            dbg_d["v"][:, kb * DG:(kb + 1) * DG], v_sb[kb])

                # ------------ phase 3: output projection ------------
                # pso tiles join the pv ring (no pool-close barrier).
                for c in range(NQC):
                    csl = slice(c * QC, (c + 1) * QC)
                    for dt in range(DT):
                        dsl = slice(dt * P, (dt + 1) * P)
                        pso = pv_ps.tile([P, QC], F32, name="pso", tag="pv")
                        for ft in range(FT):
                            nc.tensor.matmul(
                                pso,
                                lhsT=wot_sb[ft][:, dsl],
                                rhs=onorm[ft][:, csl],
                                start=ft == 0,
                                stop=ft == FT - 1,
                            )
                        o_sb = o_sb_pool.tile([P, QC], BF16, name="o_sb",
                                              tag="osb")
                        nc.vector.tensor_copy(o_sb, pso)
                        nc.sync.dma_start(out_d[dsl, csl], o_sb)

    return nc


_CACHE: dict = {}


def _get_nc(seq: int = S) -> bass.Bass:
    key = f"nc{seq}"
    if key not in _CACHE:
        nc = build_attention_nc(seq)
        nc.finalize()  # runs Bacc.compile(): reg alloc + wait legalization
        _CACHE[key] = nc
    return _CACHE[key]


def make_in_maps(x, Wq, bq, Wk, Wv, bv, Wo, seq: int = S):
    bf = ml_dtypes.bfloat16
    scale = 1.0 / (SC * math.sqrt(DK))
    x = np.asarray(x, np.float32)
    Wq = np.asarray(Wq, np.float32)
    bq = np.asarray(bq, np.float32)
    Wk = np.asarray(Wk, np.float32)
    Wv = np.asarray(Wv, np.float32)
    bv = np.asarray(bv, np.float32)
    Wo = np.asarray(Wo, np.float32)
    f8 = ml_dtypes.float8_e4m3
    in_maps = []
    for core in range(NCORES):
        b, g = divmod(core, GROUPS)
        gsl = slice(g * DG, (g + 1) * DG)
        xt = np.ascontiguousarray(x[b, :seq, :].T)
        in_maps.append(
            {
                "xt": xt.astype(bf),
                "xt8": xt.astype(f8),
                "wq8": np.ascontiguousarray(
                    (Wq[gsl, :] * WBOOST).T).astype(f8),
                "wk8": np.ascontiguousarray(
                    (Wk[gsl, :] * WBOOST).T).astype(f8),
                "wvt": np.ascontiguousarray(Wv[gsl, :].T).astype(bf),
                "wot": np.ascontiguousarray(Wo[:, gsl].T).astype(bf),
                "bqs": np.ascontiguousarray(
                    (bq[gsl] * scale).astype(np.float32).reshape(FT, P).T
                ),
            }
        )
    return in_maps


def run_device(in_maps, seq: int = S, trace: bool = False):
    nc = _get_nc(seq)
    return run_bass_kernel_spmd(nc, in_maps, list(range(NCORES)), trace=trace)


def kernel(x, Wq, bq, Wk, bk, Wv, bv, Wo, bo):
    in_maps = make_in_maps(x, Wq, bq, Wk, Wv, bv, Wo)
    res = run_device(in_maps).results
    # bv passes through the attention average unchanged (weights sum to 1),
    # so its contribution to the output is exactly Wo @ bv, added here.
    bias = np.asarray(bo, np.float32) + np.asarray(Wo, np.float32) @ np.asarray(
        bv, np.float32
    )
    out = np.empty((B, S, D), np.float32)
    for b in range(B):
        acc = res[2 * b]["out"].astype(np.float32) + res[2 * b + 1]["out"].astype(
            np.float32
        )
        out[b] = acc.T + bias[None, :]
    return out


# revision 18
# speedup vs baseline: 1.0898x; 1.0898x over previous
"""Multi-head self-attention on 8 Trainium2 NeuronCores.

Problem: B=4, S=2048, D=1024, H=16 heads (dk=64), torch-Linear style
projections (y = x @ W.T + b), softmax attention, output projection.

Sharding: 8 cores = 4 batches x 2 head-groups (8 heads each).  Each core
computes, for its (batch b, group g):
    QT = (Wq_g/(4*sqrt(dk))) @ x_b.T + bq_g/(4*sqrt(dk))  [512, S]
         (scores are produced pre-divided by 4; the exp stage multiplies
          back: ACT uses Exp scale=4, DVE evaluates p(u)^4 with p ~ e^u)
    KT = Wk_g @ x_b.T            [512, S]  (bk dropped: it shifts scores
                                  uniformly per query, cancels in softmax)
    V  = x_b @ Wv_g.T            [S, 512]  (keys on partitions; bv is folded
                                  into the host bias as Wo @ bv since the
                                  softmax weights sum to one)
    per head pair (h0, h1): scoresT = K_h @ Q_h.T  [S(keys), S(queries)]
      - the two heads' score matmuls are 64-contraction row-tiled and run
        CONCURRENTLY in the PE array (rows 0:63 / 64:127)
    expT = exp(scoresT)          (no max-subtraction: |scores| < ~3.5)
        - scalar engine (ACT) exponentiates head h0's scores (Exp, scale=4)
        - vector engine (DVE) head h1's, via one custom micro-coded op
          p(u)^4, p = deg-3 rel-minimax of e^u
    PV via COL-TILED concurrent pairs: stationary V_h0 (64 cols) at
      tile_position (0,0) and V_h1 at (0,64) write one PSUM bank holding
      [outT_h0 ; outT_h1] row-aligned; an identical ones[128,64] stationary
      pair writes a second bank with [den_h0 ; den_h1].  One DVE divide
      (pv / den) then normalizes BOTH heads at once -- no cross-partition
      swap, no reciprocal+mult chain.
    partialT = Wo_g @ onorm_all  [1024, S] in bf16
Host sums the two group partials per batch, transposes, adds bo + Wo@bv.

PE mode switches (row-tiled scores <-> col-tiled PV/DEN) drain the array
(~100ns); ticks are emitted in BATCHES OF TWO so each batch pays only two
mode transitions: [S(t) S(t+1)] [PV/DEN(t-3) PV/DEN(t-2)].

Device dtypes: bf16 matmul operands, f32 PSUM/exp/normalization, bf16 out.
"""

import math

import numpy as np
import ml_dtypes

import concourse.bass as bass
import concourse.bacc as bacc_mod
import concourse.mybir as mybir
import concourse.tile as tile
from concourse.bass_utils import run_bass_kernel_spmd

BF16 = mybir.dt.bfloat16
F32 = mybir.dt.float32
F8E4 = mybir.dt.float8e4
AF = mybir.ActivationFunctionType
DR = mybir.MatmulPerfMode.DoubleRow

B, S, D, H = 4, 2048, 1024, 16
DK = D // H  # 64
NCORES = 8
GROUPS = 2  # tensor-parallel head groups
DG = D // GROUPS  # 512 features per group
P = 128
FT = DG // P  # 4 feature tiles per group == head pairs

# exp: scores are pre-scaled by 1/SC; ACT uses Exp(scale=SC) on head h0,
# DVE evaluates p(u)^4 on head h1 in ONE custom op with
# p = c0 + u*(c1 + u*(c2 + u*c3)) ~ e^u (c3 rides in via the Src1 spill).
SC = 4.0
# fp8 path: Q/K projections run as e4m3 DoubleRow matmuls (2 contraction
# rows per PE cell, ~1.8x).  Weights are pre-boosted by WBOOST so their
# entries sit in e4m3's normal range; the boost is divided back out in the
# PSUM->SBUF activation copy (scale=).  V / Wo stay bf16: their quantization
# noise would hit the output directly, while q/k noise only perturbs scores.
WBOOST = 64.0
EXP_C0 = 0.99773437
EXP_C1 = 1.0064234
EXP_C2 = 0.5302388
EXP_C3 = 0.16039258

# fused divide out = in0/in1 via the BITWISE_NOT exponent-flip seed:
# 1/b = ~b · g(t) with t = b·~b ∈ [-4.5, -4]; g = deg-2 rel-minimax of 1/t
# (max rel err 5.1e-5).  out = (in0·~b)·(c0 + t·(c1 + t·c2)) — 8 ALU ops,
# exactly the DVE slice budget, one instruction.
DIV_C0 = -0.7071066070499812
DIV_C1 = -0.16652203635848312
DIV_C2 = -0.013060548941608689

_DVE_OPS = {}
DEBUG_DUMP = False


def _register_dve_exp_ops():
    """Register the custom DVE micro-ops (exp path + fused divide).
    Idempotent; appends to concourse's module-level op registry."""
    if _DVE_OPS:
        return _DVE_OPS
    from concourse.dve_spec import (
        Spec, Src0, Src1, C0, C1, C2, C3, Bin, AluOp, sq, lower, _has_src1,
        _spill_c3_to_src1,
    )
    from concourse.dve_uop import DveOpSpec
    from concourse.dve_ops import (
        DveOp,
        OPS,
        CUSTOM_DVE_SPECS,
        _SUB_OPCODE_FOR_NAME,
        _CUSTOM_DVE_ROW_BASE,
    )

    u = Src0

    def _ref_exp4p(in0, in1, s0, s1, imm2):
        p = s0 + in0 * (s1 + in0 * (imm2 + in0 * in1))
        return (p * p) ** 2

    _notb = Bin(AluOp.BITWISE_NOT, Src1, Src1)
    _t = Src1 * _notb

    def _ref_divab(in0, in1, s0, s1, imm2):
        import numpy as np

        notb = (~in1.view(np.int32)).view(np.float32)
        t = in1 * notb
        return (in0 * notb) * (s0 + t * (s1 + t * imm2))

    specs = {
        "ANT_EXP4P": Spec(
            body=_spill_c3_to_src1(sq(sq(C0 + u * (C1 + u * (C2 + u * C3))))),
            reference=_ref_exp4p,
        ),
        "ANT_DIVAB": Spec(
            body=(Src0 * _notb) * (C0 + _t * (C1 + _t * C2)),
            reference=_ref_divab,
        ),
    }
    for name, sp in specs.items():
        if name not in _SUB_OPCODE_FOR_NAME:
            row = _CUSTOM_DVE_ROW_BASE + len(OPS)
            op = DveOp(name, sp, subdim=False, uops_sha={})
            _SUB_OPCODE_FOR_NAME[name] = row
            OPS.append(op)
            CUSTOM_DVE_SPECS[name] = sp
            for ver in ("v3", "v4"):
                s = DveOpSpec(
                    name=name, opcode=row, uops=lower(sp, ver=ver),
                    rd1_en=_has_src1(sp),
                )
                op.uops_sha[ver] = s.sha(ver)
        else:
            op = next(o for o in OPS if o.name == name)
        _DVE_OPS[name] = op
    return _DVE_OPS


def build_attention_nc(seq: int = S) -> bass.Bass:
    KB = seq // P  # key blocks
    DKB = D // P  # 8 contraction blocks for projections
    QH = min(512, seq)  # query stripe processed per attention pass
    NQH = seq // QH
    QC = min(512, QH)  # matmul moving-operand chunk
    NQC = seq // QC  # chunks per full seq
    DT = D // P
    XC = 512  # x DMA column-chunk width
    NXC = seq // XC

    ops = _register_dve_exp_ops()
    exp4p = ops["ANT_EXP4P"]
    divab = ops["ANT_DIVAB"]

    nc = bacc_mod.Bacc("TRN2", num_devices=NCORES)
    xt_d = nc.declare_dram_parameter("xt", [D, seq], BF16, isOutput=False)
    wqt_d = nc.declare_dram_parameter("wqt", [D, DG], BF16, isOutput=False)
    wkt_d = nc.declare_dram_parameter("wkt", [D, DG], BF16, isOutput=False)
    wvt_d = nc.declare_dram_parameter("wvt", [D, DG], BF16, isOutput=False)
    wot_d = nc.declare_dram_parameter("wot", [DG, D], BF16, isOutput=False)
    bq_d = nc.declare_dram_parameter("bqs", [P, FT], F32, isOutput=False)
    out_d = nc.declare_dram_parameter("out", [D, seq], BF16, isOutput=True)
    dbg_d = None
    if DEBUG_DUMP:
        dbg_d = {
            name: nc.declare_dram_parameter(f"dbg_{name}", [P, FT * seq], BF16,
                                            isOutput=True)
            for name in ("qt", "kt", "onorm")
        }
        dbg_d["v"] = nc.declare_dram_parameter("dbg_v", [P, seq // P * DG],
                                               BF16, isOutput=True)

    with tile.TileContext(nc) as tc:
        with tc.tile_pool(name="persist", bufs=1) as persist:
            bq_sb = persist.tile([P, FT], F32, name="bq_sb")
            nc.sync.dma_start(bq_sb, bq_d[:, :])
            c3_sb = persist.tile([P, 1], F32, name="c3_sb")
            nc.vector.memset(c3_sb, EXP_C3)

            qt_sb = [persist.tile([P, seq], BF16, name=f"qt{i}") for i in range(FT)]
            kt_sb = [persist.tile([P, seq], BF16, name=f"kt{i}") for i in range(FT)]
            # plain V tiles: [128 keys, 512 features] per key block
            v_sb = [persist.tile([P, DG], BF16, name=f"v{i}") for i in range(KB)]
            ones_sb = persist.tile([P, DK], BF16, name="ones_sb")
            nc.vector.memset(ones_sb, 1.0)
            wot_sb = [persist.tile([P, D], BF16, name=f"wot{i}") for i in range(FT)]
            onorm = [persist.tile([P, seq], BF16, name=f"onorm{i}") for i in range(FT)]

            # ---------------- phase 1: projections ----------------
            with (
                tc.tile_pool(name="xw", bufs=1) as xw_pool,
                tc.tile_pool(name="pps", bufs=4, space="PSUM") as proj_ps,
            ):
                # Slab tiles: each holds all 8 contraction sub-tiles side by
                # side so ONE dma_start loads a whole slab.
                xt_all = xw_pool.tile([P, DKB * seq], BF16, name="xts")
                wq_all = xw_pool.tile([P, DKB * DG], BF16, name="wqts")
                wk_all = xw_pool.tile([P, DKB * DG], BF16, name="wkts")
                wv_all = xw_pool.tile([P, DKB * DG], BF16, name="wvts")
                xt_sb = [xt_all[:, i * seq : (i + 1) * seq] for i in range(DKB)]
                wqt_sb = [wq_all[:, i * DG : (i + 1) * DG] for i in range(DKB)]
                wkt_sb = [wk_all[:, i * DG : (i + 1) * DG] for i in range(DKB)]
                wvt_sb = [wv_all[:, i * DG : (i + 1) * DG] for i in range(DKB)]

                def wdma(dst_all, w_d):
                    return (
                        dst_all.rearrange("p (k g) -> p k g", g=DG),
                        w_d.rearrange("(k p) g -> p k g", p=P),
                    )

                def xsrc(c):
                    csl = slice(c * XC, (c + 1) * XC)
                    return (
                        xt_all.rearrange("p (k s) -> p k s", s=seq)[:, :, csl],
                        xt_d.rearrange("(k p) s -> p k s", p=P)[:, :, csl],
                    )

                xd0, xs0 = xsrc(0)
                nc.sync.dma_start(xd0[:, 0:4, :], xs0[:, 0:4, :])
                nc.sync.dma_start(xd0[:, 4:8, :], xs0[:, 4:8, :])
                wd, ws = wdma(wq_all, wqt_d)
                nc.scalar.dma_start(wd[:, 0:4, :], ws[:, 0:4, :])
                nc.scalar.dma_start(wd[:, 4:8, :], ws[:, 4:8, :])
                nc.scalar.dma_start(*wdma(wk_all, wkt_d))
                for c in range(1, NXC):
                    (nc.sync if c % 2 else nc.scalar).dma_start(*xsrc(c))
                nc.scalar.dma_start(*wdma(wv_all, wvt_d))
                for ft in range(FT):
                    nc.sync.dma_start(wot_sb[ft], wot_d[ft * P : (ft + 1) * P, :])

                # QT / KT: features on partitions, queries on free dim.
                for c in range(NQC):
                    csl = slice(c * QC, (c + 1) * QC)
                    for ft in range(FT):
                        fsl = slice(ft * P, (ft + 1) * P)
                        psq = proj_ps.tile([P, QC], F32, name="psq", tag="proj")
                        for k in range(DKB):
                            nc.tensor.matmul(
                                psq,
                                lhsT=wqt_sb[k][:, fsl],
                                rhs=xt_sb[k][:, csl],
                                start=k == 0,
                                stop=k == DKB - 1,
                            )
                        nc.scalar.activation(
                            qt_sb[ft][:, csl], psq, AF.Identity,
                            bias=bq_sb[:, ft : ft + 1],
                        )
                    for ft in range(FT):
                        fsl = slice(ft * P, (ft + 1) * P)
                        psk = proj_ps.tile([P, QC], F32, name="psk", tag="proj")
                        for k in range(DKB):
                            nc.tensor.matmul(
                                psk,
                                lhsT=wkt_sb[k][:, fsl],
                                rhs=xt_sb[k][:, csl],
                                start=k == 0,
                                stop=k == DKB - 1,
                            )
                        nc.vector.tensor_copy(kt_sb[ft][:, csl], psk)

                # V: keys on partitions, features on free dim.  bv is folded
                # into the host-side bias.  Plain contiguous copy (scalar
                # engine; idle here).
                for kb in range(KB):
                    ksl = slice(kb * P, (kb + 1) * P)
                    psv = proj_ps.tile([P, DG], F32, name="psv", tag="proj")
                    for k in range(DKB):
                        nc.tensor.matmul(
                            psv,
                            lhsT=xt_sb[k][:, ksl],
                            rhs=wvt_sb[k],
                            start=k == 0,
                            stop=k == DKB - 1,
                        )
                    nc.scalar.activation(v_sb[kb], psv, AF.Copy)

            # ---------------- phase 2: attention ----------------
            with (
                tc.tile_pool(name="sps", bufs=2, space="PSUM") as s_ps,
                tc.tile_pool(name="pvps", bufs=2, space="PSUM") as pv_ps,
                tc.tile_pool(name="epool", bufs=6) as e_pool,
                tc.tile_pool(name="dpool", bufs=2) as den_pool,
                tc.tile_pool(name="osb", bufs=4) as o_sb_pool,
            ):
                # Ticks t = (qh, pr, kb) linearized; emitted in batches of
                # two so the PE pays one row->col and one col->row mode
                # drain per TWO ticks.  PV/DEN for tick t are emitted at
                # batch containing t+DEPTH.
                DEPTH = 4  # must be EVEN so (t-DEPTH, t-DEPTH+1) aligns
                # with tick-pair boundaries (kb0's start=True first)
                TICKS = NQH * FT * KB

                def tick_coords(t):
                    qh, r = divmod(t, FT * KB)
                    pr, kb = divmod(r, KB)
                    return qh, pr, kb

                epipe = {}  # t -> (pv, den, pr, kb, e0, e1)
                pend = []  # (due_batch, closure) for norm ops
                pvden = {}  # pr-slot -> (pv, den) current accumulators

                def emit_scores(t):
                    qh, pr, kb = tick_coords(t)
                    qsl = slice(qh * QH, (qh + 1) * QH)
                    ksl = slice(kb * P, (kb + 1) * P)
                    s0 = s_ps.tile([P, QH], F32, name="s0", tag="s0")
                    s1 = s_ps.tile([P, QH], F32, name="s1", tag="s1")
                    nc.tensor.matmul(
                        s0,
                        lhsT=kt_sb[pr][0:64, ksl],
                        rhs=qt_sb[pr][0:64, qsl],
                        start=True, stop=True,
                    )
                    nc.tensor.matmul(
                        s1,
                        lhsT=kt_sb[pr][64:128, ksl],
                        rhs=qt_sb[pr][64:128, qsl],
                        start=True, stop=True,
                    )
                    e0 = e_pool.tile([P, QH], BF16, name="e0", tag="e0")
                    e1 = e_pool.tile([P, QH], BF16, name="e1", tag="e1")
                    nc.scalar.activation(e0, s0, AF.Exp, scale=SC)
                    nc.vector._custom_dve(
                        exp4p, out=e1, in0=s1, in1=c3_sb[:, 0:1],
                        s0=EXP_C0, s1=EXP_C1, imm2=EXP_C2,
                    )
                    if kb == 0:
                        pv = pv_ps.tile([P, QH], F32, name="pv", tag="pv")
                        den = pv_ps.tile([P, QH], F32, name="den", tag="den")
                        pvden[t // KB] = (pv, den)
                    pv, den = pvden[t // KB]
                    epipe[t] = (pv, den, pr, kb, e0, e1)

                def emit_pvden(t):
                    pv, den, pr, kb, e0, e1 = epipe.pop(t)
                    vc = slice(pr * P, pr * P + 64)
                    vc1 = slice(pr * P + 64, (pr + 1) * P)
                    nc.tensor.matmul(
                        pv[0:64, :], lhsT=v_sb[kb][:, vc], rhs=e0,
                        start=kb == 0, stop=kb == KB - 1,
                        tile_position=(0, 0),
                    )
                    nc.tensor.matmul(
                        pv[64:128, :], lhsT=v_sb[kb][:, vc1], rhs=e1,
                        start=kb == 0, stop=kb == KB - 1,
                        tile_position=(0, 64),
                    )
                    nc.tensor.matmul(
                        den[0:64, :], lhsT=ones_sb, rhs=e0,
                        start=kb == 0, stop=kb == KB - 1,
                        tile_position=(0, 0),
                    )
                    nc.tensor.matmul(
                        den[64:128, :], lhsT=ones_sb, rhs=e1,
                        start=kb == 0, stop=kb == KB - 1,
                        tile_position=(0, 64),
                    )
                    if kb == KB - 1:
                        qh, pr_, _ = tick_coords(t)
                        qsl = slice(qh * QH, (qh + 1) * QH)

                        # DVE may read only ONE operand from PSUM: the
                        # scalar engine (same act table as Exp, no table
                        # reload) first copies den to SBUF, then ONE custom
                        # DVE fused-divide normalizes both heads at once.
                        state = {}

                        def norm_copy(den=den, state=state):
                            den_sb = den_pool.tile([P, QH], F32,
                                                   name="den_sb", tag="den_sb")
                            nc.scalar.activation(den_sb, den, AF.Copy)
                            state["den_sb"] = den_sb

                        def norm_div(pv=pv, pr_=pr_, qsl=qsl, state=state):
                            nc.vector._custom_dve(
                                divab, out=onorm[pr_][:, qsl], in0=pv,
                                in1=state["den_sb"],
                                s0=DIV_C0, s1=DIV_C1, imm2=DIV_C2,
                            )

                        pend.append((t // 2 + 2, norm_copy))
                        pend.append((t // 2 + 3, norm_div))

                def flush_due(bi):
                    while pend and pend[0][0] <= bi:
                        pend.pop(0)[1]()
                    pend.sort(key=lambda e: e[0])

                for bi in range(TICKS // 2):
                    t0 = 2 * bi
                    emit_scores(t0)
                    emit_scores(t0 + 1)
                    c0 = t0 - DEPTH  # always odd-aligned pair (c0, c0+1)
                    if c0 >= 0:
                        emit_pvden(c0)
                        emit_pvden(c0 + 1)
                    flush_due(bi)
                # drain the pipe
                for t in sorted(epipe.keys()):
                    emit_pvden(t)
                flush_due(1 << 30)
                while pend:
                    pend.pop(0)[1]()

                if DEBUG_DUMP:
                    for i in range(FT):
                        nc.sync.dma_start(
                            dbg_d["qt"][:, i * seq:(i + 1) * seq], qt_sb[i])
                        nc.sync.dma_start(
                            dbg_d["kt"][:, i * seq:(i + 1) * seq], kt_sb[i])
                        nc.sync.dma_start(
                            dbg_d["onorm"][:, i * seq:(i + 1) * seq], onorm[i])
                    for kb in range(KB):
                        nc.sync.dma_start(
                            dbg_d["v"][:, kb * DG:(kb + 1) * DG], v_sb[kb])

                # ------------ phase 3: output projection ------------
                # pso tiles join the pv ring (no pool-close barrier).
                for c in range(NQC):
                    csl = slice(c * QC, (c + 1) * QC)
                    for dt in range(DT):
                        dsl = slice(dt * P, (dt + 1) * P)
                        pso = pv_ps.tile([P, QC], F32, name="pso", tag="pv")
                        for ft in range(FT):
                            nc.tensor.matmul(
                                pso,
                                lhsT=wot_sb[ft][:, dsl],
                                rhs=onorm[ft][:, csl],
                                start=ft == 0,
                                stop=ft == FT - 1,
                            )
                        o_sb = o_sb_pool.tile([P, QC], BF16, name="o_sb",
                                              tag="osb")
                        nc.vector.tensor_copy(o_sb, pso)
                        nc.sync.dma_start(out_d[dsl, csl], o_sb)

    return nc


_CACHE: dict = {}


def _get_nc(seq: int = S) -> bass.Bass:
    key = f"nc{seq}"
    if key not in _CACHE:
        nc = build_attention_nc(seq)
        nc.finalize()  # runs Bacc.compile(): reg alloc + wait legalization
        _CACHE[key] = nc
    return _CACHE[key]


def make_in_maps(x, Wq, bq, Wk, Wv, bv, Wo, seq: int = S):
    bf = ml_dtypes.bfloat16
    scale = 1.0 / (SC * math.sqrt(DK))
    x = np.asarray(x, np.float32)
    Wq = np.asarray(Wq, np.float32)
    bq = np.asarray(bq, np.float32)
    Wk = np.asarray(Wk, np.float32)
    Wv = np.asarray(Wv, np.float32)
    bv = np.asarray(bv, np.float32)
    Wo = np.asarray(Wo, np.float32)
    in_maps = []
    for core in range(NCORES):
        b, g = divmod(core, GROUPS)
        gsl = slice(g * DG, (g + 1) * DG)
        in_maps.append(
            {
                "xt": np.ascontiguousarray(x[b, :seq, :].T).astype(bf),
                "wqt": np.ascontiguousarray((Wq[gsl, :] * scale).T).astype(bf),
                "wkt": np.ascontiguousarray(Wk[gsl, :].T).astype(bf),
                "wvt": np.ascontiguousarray(Wv[gsl, :].T).astype(bf),
                "wot": np.ascontiguousarray(Wo[:, gsl].T).astype(bf),
                "bqs": np.ascontiguousarray(
                    (bq[gsl] * scale).astype(np.float32).reshape(FT, P).T
                ),
            }
        )
    return in_maps


def run_device(in_maps, seq: int = S, trace: bool = False):
    nc = _get_nc(seq)
    return run_bass_kernel_spmd(nc, in_maps, list(range(NCORES)), trace=trace)


def kernel(x, Wq, bq, Wk, bk, Wv, bv, Wo, bo):
    in_maps = make_in_maps(x, Wq, bq, Wk, Wv, bv, Wo)
    res = run_device(in_maps).results
    # bv passes through the attention average unchanged (weights sum to 1),
    # so its contribution to the output is exactly Wo @ bv, added here.
    bias = np.asarray(bo, np.float32) + np.asarray(Wo, np.float32) @ np.asarray(
        bv, np.float32
    )
    out = np.empty((B, S, D), np.float32)
    for b in range(B):
        acc = res[2 * b]["out"].astype(np.float32) + res[2 * b + 1]["out"].astype(
            np.float32
        )
        out[b] = acc.T + bias[None, :]
    return out


# revision 19
# speedup vs baseline: 1.0957x; 1.0054x over previous
"""Multi-head self-attention on 8 Trainium2 NeuronCores.

Problem: B=4, S=2048, D=1024, H=16 heads (dk=64), torch-Linear style
projections (y = x @ W.T + b), softmax attention, output projection.

Sharding: 8 cores = 4 batches x 2 head-groups (8 heads each).  Each core
computes, for its (batch b, group g):
    QT = (Wq_g/(4*sqrt(dk))) @ x_b.T + bq_g/(4*sqrt(dk))  [512, S]
         (scores are produced pre-divided by 4; the exp stage multiplies
          back: ACT uses Exp scale=4, DVE evaluates p(u)^4 with p ~ e^u)
    KT = Wk_g @ x_b.T            [512, S]  (bk dropped: it shifts scores
                                  uniformly per query, cancels in softmax)
    V  = x_b @ Wv_g.T            [S, 512]  (keys on partitions; bv is folded
                                  into the host bias as Wo @ bv since the
                                  softmax weights sum to one)
    per head pair (h0, h1): scoresT = K_h @ Q_h.T  [S(keys), S(queries)]
      - the two heads' score matmuls are 64-contraction row-tiled and run
        CONCURRENTLY in the PE array (rows 0:63 / 64:127)
    expT = exp(scoresT)          (no max-subtraction: |scores| < ~3.5)
        - scalar engine (ACT) exponentiates head h0's scores (Exp, scale=4)
        - vector engine (DVE) head h1's, via one custom micro-coded op
          p(u)^4, p = deg-3 rel-minimax of e^u
    PV via COL-TILED concurrent pairs: stationary V_h0 (64 cols) at
      tile_position (0,0) and V_h1 at (0,64) write one PSUM bank holding
      [outT_h0 ; outT_h1] row-aligned; an identical ones[128,64] stationary
      pair writes a second bank with [den_h0 ; den_h1].  One DVE divide
      (pv / den) then normalizes BOTH heads at once -- no cross-partition
      swap, no reciprocal+mult chain.
    partialT = Wo_g @ onorm_all  [1024, S] in bf16
Host sums the two group partials per batch, transposes, adds bo + Wo@bv.

PE mode switches (row-tiled scores <-> col-tiled PV/DEN) drain the array
(~100ns); ticks are emitted in BATCHES OF TWO so each batch pays only two
mode transitions: [S(t) S(t+1)] [PV/DEN(t-3) PV/DEN(t-2)].

Device dtypes: bf16 matmul operands, f32 PSUM/exp/normalization, bf16 out.
"""

import math

import numpy as np
import ml_dtypes

import concourse.bass as bass
import concourse.bacc as bacc_mod
import concourse.mybir as mybir
import concourse.tile as tile
from concourse.bass_utils import run_bass_kernel_spmd

BF16 = mybir.dt.bfloat16
F32 = mybir.dt.float32
F8E4 = mybir.dt.float8e4
AF = mybir.ActivationFunctionType
DR = mybir.MatmulPerfMode.DoubleRow

B, S, D, H = 4, 2048, 1024, 16
DK = D // H  # 64
NCORES = 8
GROUPS = 2  # tensor-parallel head groups
DG = D // GROUPS  # 512 features per group
P = 128
FT = DG // P  # 4 feature tiles per group == head pairs

# exp: scores are pre-scaled by 1/SC; ACT uses Exp(scale=SC) on head h0,
# DVE evaluates p(u)^4 on head h1 in ONE custom op with
# p = c0 + u*(c1 + u*(c2 + u*c3)) ~ e^u (c3 rides in via the Src1 spill).
SC = 4.0
# fp8 path: Q/K projections run as e4m3 DoubleRow matmuls (2 contraction
# rows per PE cell, ~1.8x).  Weights are pre-boosted by WBOOST so their
# entries sit in e4m3's normal range; the boost is divided back out in the
# PSUM->SBUF activation copy (scale=).  V / Wo stay bf16: their quantization
# noise would hit the output directly, while q/k noise only perturbs scores.
WBOOST = 64.0
EXP_C0 = 0.99773437
EXP_C1 = 1.0064234
EXP_C2 = 0.5302388
EXP_C3 = 0.16039258

# fused divide out = in0/in1 via the BITWISE_NOT exponent-flip seed:
# 1/b = ~b · g(t) with t = b·~b ∈ [-4.5, -4]; g = deg-2 rel-minimax of 1/t
# (max rel err 5.1e-5).  out = (in0·~b)·(c0 + t·(c1 + t·c2)) — 8 ALU ops,
# exactly the DVE slice budget, one instruction.
DIV_C0 = -0.7071066070499812
DIV_C1 = -0.16652203635848312
DIV_C2 = -0.013060548941608689

_DVE_OPS = {}
DEBUG_DUMP = False


def _register_dve_exp_ops():
    """Register the custom DVE micro-ops (exp path + fused divide).
    Idempotent; appends to concourse's module-level op registry."""
    if _DVE_OPS:
        return _DVE_OPS
    from concourse.dve_spec import (
        Spec, Src0, Src1, C0, C1, C2, C3, Bin, AluOp, sq, lower, _has_src1,
        _spill_c3_to_src1,
    )
    from concourse.dve_uop import DveOpSpec
    from concourse.dve_ops import (
        DveOp,
        OPS,
        CUSTOM_DVE_SPECS,
        _SUB_OPCODE_FOR_NAME,
        _CUSTOM_DVE_ROW_BASE,
    )

    u = Src0

    def _ref_exp4p(in0, in1, s0, s1, imm2):
        p = s0 + in0 * (s1 + in0 * (imm2 + in0 * in1))
        return (p * p) ** 2

    _notb = Bin(AluOp.BITWISE_NOT, Src1, Src1)
    _t = Src1 * _notb

    def _ref_divab(in0, in1, s0, s1, imm2):
        import numpy as np

        notb = (~in1.view(np.int32)).view(np.float32)
        t = in1 * notb
        return (in0 * notb) * (s0 + t * (s1 + t * imm2))

    specs = {
        "ANT_EXP4P": Spec(
            body=_spill_c3_to_src1(sq(sq(C0 + u * (C1 + u * (C2 + u * C3))))),
            reference=_ref_exp4p,
        ),
        "ANT_DIVAB": Spec(
            body=(Src0 * _notb) * (C0 + _t * (C1 + _t * C2)),
            reference=_ref_divab,
        ),
    }
    for name, sp in specs.items():
        if name not in _SUB_OPCODE_FOR_NAME:
            row = _CUSTOM_DVE_ROW_BASE + len(OPS)
            op = DveOp(name, sp, subdim=False, uops_sha={})
            _SUB_OPCODE_FOR_NAME[name] = row
            OPS.append(op)
            CUSTOM_DVE_SPECS[name] = sp
            for ver in ("v3", "v4"):
                s = DveOpSpec(
                    name=name, opcode=row, uops=lower(sp, ver=ver),
                    rd1_en=_has_src1(sp),
                )
                op.uops_sha[ver] = s.sha(ver)
        else:
            op = next(o for o in OPS if o.name == name)
        _DVE_OPS[name] = op
    return _DVE_OPS


def build_attention_nc(seq: int = S) -> bass.Bass:
    KB = seq // P  # key blocks
    DKB = D // P  # 8 contraction blocks for projections
    QH = min(512, seq)  # query stripe processed per attention pass
    NQH = seq // QH
    QC = min(512, QH)  # matmul moving-operand chunk
    NQC = seq // QC  # chunks per full seq
    DT = D // P
    XC = 512  # x DMA column-chunk width
    NXC = seq // XC

    ops = _register_dve_exp_ops()
    exp4p = ops["ANT_EXP4P"]
    divab = ops["ANT_DIVAB"]

    nc = bacc_mod.Bacc("TRN2", num_devices=NCORES)
    xt_d = nc.declare_dram_parameter("xt", [D, seq], BF16, isOutput=False)
    wqt_d = nc.declare_dram_parameter("wqt", [D, DG], BF16, isOutput=False)
    wkt_d = nc.declare_dram_parameter("wkt", [D, DG], BF16, isOutput=False)
    wvt_d = nc.declare_dram_parameter("wvt", [D, DG], BF16, isOutput=False)
    wot_d = nc.declare_dram_parameter("wot", [DG, D], BF16, isOutput=False)
    bq_d = nc.declare_dram_parameter("bqs", [P, FT], F32, isOutput=False)
    out_d = nc.declare_dram_parameter("out", [D, seq], BF16, isOutput=True)
    dbg_d = None
    if DEBUG_DUMP:
        dbg_d = {
            name: nc.declare_dram_parameter(f"dbg_{name}", [P, FT * seq], BF16,
                                            isOutput=True)
            for name in ("qt", "kt", "onorm")
        }
        dbg_d["v"] = nc.declare_dram_parameter("dbg_v", [P, seq // P * DG],
                                               BF16, isOutput=True)

    with tile.TileContext(nc) as tc:
        with tc.tile_pool(name="persist", bufs=1) as persist:
            bq_sb = persist.tile([P, FT], F32, name="bq_sb")
            nc.sync.dma_start(bq_sb, bq_d[:, :])
            c3_sb = persist.tile([P, 1], F32, name="c3_sb")
            nc.vector.memset(c3_sb, EXP_C3)

            qt_sb = [persist.tile([P, seq], BF16, name=f"qt{i}") for i in range(FT)]
            kt_sb = [persist.tile([P, seq], BF16, name=f"kt{i}") for i in range(FT)]
            # plain V tiles: [128 keys, 512 features] per key block
            v_sb = [persist.tile([P, DG], BF16, name=f"v{i}") for i in range(KB)]
            ones_sb = persist.tile([P, DK], BF16, name="ones_sb")
            nc.vector.memset(ones_sb, 1.0)
            wot_sb = [persist.tile([P, D], BF16, name=f"wot{i}") for i in range(FT)]
            onorm = [persist.tile([P, seq], BF16, name=f"onorm{i}") for i in range(FT)]

            # ---------------- phase 1: projections ----------------
            with (
                tc.tile_pool(name="xw", bufs=1) as xw_pool,
                tc.tile_pool(name="pps", bufs=4, space="PSUM") as proj_ps,
            ):
                # Slab tiles: each holds all 8 contraction sub-tiles side by
                # side so ONE dma_start loads a whole slab.
                xt_all = xw_pool.tile([P, DKB * seq], BF16, name="xts")
                wq_all = xw_pool.tile([P, DKB * DG], BF16, name="wqts")
                wk_all = xw_pool.tile([P, DKB * DG], BF16, name="wkts")
                wv_all = xw_pool.tile([P, DKB * DG], BF16, name="wvts")
                xt_sb = [xt_all[:, i * seq : (i + 1) * seq] for i in range(DKB)]
                wqt_sb = [wq_all[:, i * DG : (i + 1) * DG] for i in range(DKB)]
                wkt_sb = [wk_all[:, i * DG : (i + 1) * DG] for i in range(DKB)]
                wvt_sb = [wv_all[:, i * DG : (i + 1) * DG] for i in range(DKB)]

                def wdma(dst_all, w_d):
                    return (
                        dst_all.rearrange("p (k g) -> p k g", g=DG),
                        w_d.rearrange("(k p) g -> p k g", p=P),
                    )

                def xsrc(c):
                    csl = slice(c * XC, (c + 1) * XC)
                    return (
                        xt_all.rearrange("p (k s) -> p k s", s=seq)[:, :, csl],
                        xt_d.rearrange("(k p) s -> p k s", p=P)[:, :, csl],
                    )

                xd0, xs0 = xsrc(0)
                nc.sync.dma_start(xd0[:, 0:4, :], xs0[:, 0:4, :])
                nc.sync.dma_start(xd0[:, 4:8, :], xs0[:, 4:8, :])
                wd, ws = wdma(wq_all, wqt_d)
                nc.scalar.dma_start(wd[:, 0:4, :], ws[:, 0:4, :])
                nc.scalar.dma_start(wd[:, 4:8, :], ws[:, 4:8, :])
                nc.scalar.dma_start(*wdma(wk_all, wkt_d))
                for c in range(1, NXC):
                    (nc.sync if c % 2 else nc.scalar).dma_start(*xsrc(c))
                nc.scalar.dma_start(*wdma(wv_all, wvt_d))
                for ft in range(FT):
                    nc.sync.dma_start(wot_sb[ft], wot_d[ft * P : (ft + 1) * P, :])

                # QT / KT: features on partitions, queries on free dim.
                for c in range(NQC):
                    csl = slice(c * QC, (c + 1) * QC)
                    for ft in range(FT):
                        fsl = slice(ft * P, (ft + 1) * P)
                        psq = proj_ps.tile([P, QC], F32, name="psq", tag="proj")
                        for k in range(DKB):
                            nc.tensor.matmul(
                                psq,
                                lhsT=wqt_sb[k][:, fsl],
                                rhs=xt_sb[k][:, csl],
                                start=k == 0,
                                stop=k == DKB - 1,
                            )
                        nc.scalar.activation(
                            qt_sb[ft][:, csl], psq, AF.Identity,
                            bias=bq_sb[:, ft : ft + 1],
                        )
                    for ft in range(FT):
                        fsl = slice(ft * P, (ft + 1) * P)
                        psk = proj_ps.tile([P, QC], F32, name="psk", tag="proj")
                        for k in range(DKB):
                            nc.tensor.matmul(
                                psk,
                                lhsT=wkt_sb[k][:, fsl],
                                rhs=xt_sb[k][:, csl],
                                start=k == 0,
                                stop=k == DKB - 1,
                            )
                        nc.vector.tensor_copy(kt_sb[ft][:, csl], psk)

                # V: keys on partitions, features on free dim.  bv is folded
                # into the host-side bias.  Plain contiguous copy (scalar
                # engine; idle here).
                for kb in range(KB):
                    ksl = slice(kb * P, (kb + 1) * P)
                    psv = proj_ps.tile([P, DG], F32, name="psv", tag="proj")
                    for k in range(DKB):
                        nc.tensor.matmul(
                            psv,
                            lhsT=xt_sb[k][:, ksl],
                            rhs=wvt_sb[k],
                            start=k == 0,
                            stop=k == DKB - 1,
                        )
                    nc.scalar.activation(v_sb[kb], psv, AF.Copy)

            # ---------------- phase 2: attention ----------------
            with (
                tc.tile_pool(name="sps", bufs=2, space="PSUM") as s_ps,
                tc.tile_pool(name="pvps", bufs=2, space="PSUM") as pv_ps,
                tc.tile_pool(name="epool", bufs=6) as e_pool,
                tc.tile_pool(name="dpool", bufs=2) as den_pool,
                tc.tile_pool(name="osb", bufs=4) as o_sb_pool,
            ):
                # Ticks t = (qh, pr, kb) linearized; emitted in batches of
                # two so the PE pays one row->col and one col->row mode
                # drain per TWO ticks.  PV/DEN for tick t are emitted at
                # batch containing t+DEPTH.
                DEPTH = 4  # must be EVEN so (t-DEPTH, t-DEPTH+1) aligns
                # with tick-pair boundaries (kb0's start=True first)
                TICKS = NQH * FT * KB

                def tick_coords(t):
                    qh, r = divmod(t, FT * KB)
                    pr, kb = divmod(r, KB)
                    return qh, pr, kb

                epipe = {}  # t -> (pv, den, pr, kb, e0, e1)
                pend = []  # (due_batch, closure) for norm ops
                pvden = {}  # pr-slot -> (pv, den) current accumulators

                def emit_scores(t):
                    qh, pr, kb = tick_coords(t)
                    qsl = slice(qh * QH, (qh + 1) * QH)
                    ksl = slice(kb * P, (kb + 1) * P)
                    s0 = s_ps.tile([P, QH], F32, name="s0", tag="s0")
                    s1 = s_ps.tile([P, QH], F32, name="s1", tag="s1")
                    nc.tensor.matmul(
                        s0,
                        lhsT=kt_sb[pr][0:64, ksl],
                        rhs=qt_sb[pr][0:64, qsl],
                        start=True, stop=True,
                    )
                    nc.tensor.matmul(
                        s1,
                        lhsT=kt_sb[pr][64:128, ksl],
                        rhs=qt_sb[pr][64:128, qsl],
                        start=True, stop=True,
                    )
                    e0 = e_pool.tile([P, QH], BF16, name="e0", tag="e0")
                    e1 = e_pool.tile([P, QH], BF16, name="e1", tag="e1")
                    nc.scalar.activation(e0, s0, AF.Exp, scale=SC)
                    nc.vector._custom_dve(
                        exp4p, out=e1, in0=s1, in1=c3_sb[:, 0:1],
                        s0=EXP_C0, s1=EXP_C1, imm2=EXP_C2,
                    )
                    if kb == 0:
                        pv = pv_ps.tile([P, QH], F32, name="pv", tag="pv")
                        den = pv_ps.tile([P, QH], F32, name="den", tag="den")
                        pvden[t // KB] = (pv, den)
                    pv, den = pvden[t // KB]
                    epipe[t] = (pv, den, pr, kb, e0, e1)

                def emit_pvden(t):
                    pv, den, pr, kb, e0, e1 = epipe.pop(t)
                    vc = slice(pr * P, pr * P + 64)
                    vc1 = slice(pr * P + 64, (pr + 1) * P)
                    nc.tensor.matmul(
                        pv[0:64, :], lhsT=v_sb[kb][:, vc], rhs=e0,
                        start=kb == 0, stop=kb == KB - 1,
                        tile_position=(0, 0),
                    )
                    nc.tensor.matmul(
                        pv[64:128, :], lhsT=v_sb[kb][:, vc1], rhs=e1,
                        start=kb == 0, stop=kb == KB - 1,
                        tile_position=(0, 64),
                    )
                    nc.tensor.matmul(
                        den[0:64, :], lhsT=ones_sb, rhs=e0,
                        start=kb == 0, stop=kb == KB - 1,
                        tile_position=(0, 0),
                    )
                    nc.tensor.matmul(
                        den[64:128, :], lhsT=ones_sb, rhs=e1,
                        start=kb == 0, stop=kb == KB - 1,
                        tile_position=(0, 64),
                    )
                    if kb == KB - 1:
                        qh, pr_, _ = tick_coords(t)
                        qsl = slice(qh * QH, (qh + 1) * QH)

                        # DVE may read only ONE operand from PSUM: the
                        # scalar engine (same act table as Exp, no table
                        # reload) first copies den to SBUF, then ONE custom
                        # DVE fused-divide normalizes both heads at once.
                        state = {}

                        def norm_copy(den=den, state=state):
                            den_sb = den_pool.tile([P, QH], F32,
                                                   name="den_sb", tag="den_sb")
                            nc.scalar.activation(den_sb, den, AF.Copy)
                            state["den_sb"] = den_sb

                        def norm_div(pv=pv, pr_=pr_, qsl=qsl, state=state):
                            nc.vector._custom_dve(
                                divab, out=onorm[pr_][:, qsl], in0=pv,
                                in1=state["den_sb"],
                                s0=DIV_C0, s1=DIV_C1, imm2=DIV_C2,
                            )

                        pend.append((t // 2 + 2, norm_copy))
                        pend.append((t // 2 + 3, norm_div))

                def flush_due(bi):
                    while pend and pend[0][0] <= bi:
                        pend.pop(0)[1]()
                    pend.sort(key=lambda e: e[0])

                for bi in range(TICKS // 2):
                    t0 = 2 * bi
                    emit_scores(t0)
                    emit_scores(t0 + 1)
                    c0 = t0 - DEPTH  # always odd-aligned pair (c0, c0+1)
                    if c0 >= 0:
                        emit_pvden(c0)
                        emit_pvden(c0 + 1)
                    flush_due(bi)
                # drain the pipe
                for t in sorted(epipe.keys()):
                    emit_pvden(t)
                flush_due(1 << 30)
                while pend:
                    pend.pop(0)[1]()

                if DEBUG_DUMP:
                    for i in range(FT):
                        nc.sync.dma_start(
                            dbg_d["qt"][:, i * seq:(i + 1) * seq], qt_sb[i])
                        nc.sync.dma_start(
                            dbg_d["kt"][:, i * seq:(i + 1) * seq], kt_sb[i])
                        nc.sync.dma_start(
                            dbg_d["onorm"][:, i * seq:(i + 1) * seq], onorm[i])
                    for kb in range(KB):
                        nc.sync.dma_start(
                            dbg_d["v"][:, kb * DG:(kb + 1) * DG], v_sb[kb])

                # ------------ phase 3: output projection ------------
                # dt-outer, c-inner: each wot stationary is reused across
                # the 4 query chunks, so its LDWEIGHTS hides behind the
                # previous same-stationary stream (projection-phase pattern).
                # The 4 concurrent pso accumulators draw one tile from each
                # attention PSUM tag ring (8 banks total -> dt groups are
                # double-buffered, no pool-close barrier).
                PSO_TAGS = ("pv", "den", "s0", "s1")
                PSO_POOL = {"pv": pv_ps, "den": pv_ps, "s0": s_ps, "s1": s_ps}
                for dt in range(DT):
                    dsl = slice(dt * P, (dt + 1) * P)
                    pso = [
                        PSO_POOL[PSO_TAGS[c]].tile([P, QC], F32, name="pso",
                                                   tag=PSO_TAGS[c])
                        for c in range(NQC)
                    ]
                    for ft in range(FT):
                        for c in range(NQC):
                            csl = slice(c * QC, (c + 1) * QC)
                            nc.tensor.matmul(
                                pso[c],
                                lhsT=wot_sb[ft][:, dsl],
                                rhs=onorm[ft][:, csl],
                                start=ft == 0,
                                stop=ft == FT - 1,
                            )
                    for c in range(NQC):
                        csl = slice(c * QC, (c + 1) * QC)
                        o_sb = o_sb_pool.tile([P, QC], BF16, name="o_sb",
                                              tag="osb")
                        nc.vector.tensor_copy(o_sb, pso[c])
                        nc.sync.dma_start(out_d[dsl, csl], o_sb)

    return nc


_CACHE: dict = {}


def _get_nc(seq: int = S) -> bass.Bass:
    key = f"nc{seq}"
    if key not in _CACHE:
        nc = build_attention_nc(seq)
        nc.finalize()  # runs Bacc.compile(): reg alloc + wait legalization
        _CACHE[key] = nc
    return _CACHE[key]


def make_in_maps(x, Wq, bq, Wk, Wv, bv, Wo, seq: int = S):
    bf = ml_dtypes.bfloat16
    scale = 1.0 / (SC * math.sqrt(DK))
    x = np.asarray(x, np.float32)
    Wq = np.asarray(Wq, np.float32)
    bq = np.asarray(bq, np.float32)
    Wk = np.asarray(Wk, np.float32)
    Wv = np.asarray(Wv, np.float32)
    bv = np.asarray(bv, np.float32)
    Wo = np.asarray(Wo, np.float32)
    in_maps = []
    for core in range(NCORES):
        b, g = divmod(core, GROUPS)
        gsl = slice(g * DG, (g + 1) * DG)
        in_maps.append(
            {
                "xt": np.ascontiguousarray(x[b, :seq, :].T).astype(bf),
                "wqt": np.ascontiguousarray((Wq[gsl, :] * scale).T).astype(bf),
                "wkt": np.ascontiguousarray(Wk[gsl, :].T).astype(bf),
                "wvt": np.ascontiguousarray(Wv[gsl, :].T).astype(bf),
                "wot": np.ascontiguousarray(Wo[:, gsl].T).astype(bf),
                "bqs": np.ascontiguousarray(
                    (bq[gsl] * scale).astype(np.float32).reshape(FT, P).T
                ),
            }
        )
    return in_maps


def run_device(in_maps, seq: int = S, trace: bool = False):
    nc = _get_nc(seq)
    return run_bass_kernel_spmd(nc, in_maps, list(range(NCORES)), trace=trace)


def kernel(x, Wq, bq, Wk, bk, Wv, bv, Wo, bo):
    in_maps = make_in_maps(x, Wq, bq, Wk, Wv, bv, Wo)
    res = run_device(in_maps).results
    # bv passes through the attention average unchanged (weights sum to 1),
    # so its contribution to the output is exactly Wo @ bv, added here.
    bias = np.asarray(bo, np.float32) + np.asarray(Wo, np.float32) @ np.asarray(
        bv, np.float32
    )
    out = np.empty((B, S, D), np.float32)
    for b in range(B):
        acc = res[2 * b]["out"].astype(np.float32) + res[2 * b + 1]["out"].astype(
            np.float32
        )
        out[b] = acc.T + bias[None, :]
    return out


# revision 24
# speedup vs baseline: 1.1020x; 1.0058x over previous
"""Multi-head self-attention on 8 Trainium2 NeuronCores.

Problem: B=4, S=2048, D=1024, H=16 heads (dk=64), torch-Linear style
projections (y = x @ W.T + b), softmax attention, output projection.

Sharding: 8 cores = 4 batches x 2 head-groups (8 heads each).  Each core
computes, for its (batch b, group g):
    QT = (Wq_g/(4*sqrt(dk))) @ x_b.T + bq_g/(4*sqrt(dk))  [512, S]
         (scores are produced pre-divided by 4; the exp stage multiplies
          back: ACT uses Exp scale=4, DVE evaluates p(u)^4 with p ~ e^u)
    KT = Wk_g @ x_b.T            [512, S]  (bk dropped: it shifts scores
                                  uniformly per query, cancels in softmax)
    V  = x_b @ Wv_g.T            [S, 512]  (keys on partitions; bv is folded
                                  into the host bias as Wo @ bv since the
                                  softmax weights sum to one)
    per head pair (h0, h1): scoresT = K_h @ Q_h.T  [S(keys), S(queries)]
      - the two heads' score matmuls are 64-contraction row-tiled and run
        CONCURRENTLY in the PE array (rows 0:63 / 64:127)
    expT = exp(scoresT)          (no max-subtraction: |scores| < ~3.5)
        - scalar engine (ACT) exponentiates head h0's scores (Exp, scale=4)
        - vector engine (DVE) head h1's, via one custom micro-coded op
          p(u)^4, p = deg-3 rel-minimax of e^u
    PV via COL-TILED concurrent pairs: stationary V_h0 (64 cols) at
      tile_position (0,0) and V_h1 at (0,64) write one PSUM bank holding
      [outT_h0 ; outT_h1] row-aligned; an identical ones[128,64] stationary
      pair writes a second bank with [den_h0 ; den_h1].  One DVE divide
      (pv / den) then normalizes BOTH heads at once -- no cross-partition
      swap, no reciprocal+mult chain.
    partialT = Wo_g @ onorm_all  [1024, S] in bf16
Host sums the two group partials per batch, transposes, adds bo + Wo@bv.

PE mode switches (row-tiled scores <-> col-tiled PV/DEN) drain the array
(~100ns); ticks are emitted in BATCHES OF TWO so each batch pays only two
mode transitions: [S(t) S(t+1)] [PV/DEN(t-3) PV/DEN(t-2)].

Device dtypes: bf16 matmul operands, f32 PSUM/exp/normalization, bf16 out.
"""

import math

import numpy as np
import ml_dtypes

import concourse.bass as bass
import concourse.bacc as bacc_mod
import concourse.mybir as mybir
import concourse.tile as tile
from concourse.bass_utils import run_bass_kernel_spmd

BF16 = mybir.dt.bfloat16
F32 = mybir.dt.float32
F8E4 = mybir.dt.float8e4
AF = mybir.ActivationFunctionType
DR = mybir.MatmulPerfMode.DoubleRow

B, S, D, H = 4, 2048, 1024, 16
DK = D // H  # 64
NCORES = 8
GROUPS = 2  # tensor-parallel head groups
DG = D // GROUPS  # 512 features per group
P = 128
FT = DG // P  # 4 feature tiles per group == head pairs

# exp: scores are pre-scaled by 1/SC; ACT uses Exp(scale=SC) on head h0,
# DVE evaluates p(u)^4 on head h1 in ONE custom op with
# p = c0 + u*(c1 + u*(c2 + u*c3)) ~ e^u (c3 rides in via the Src1 spill).
SC = 4.0
# fp8 path: Q/K projections run as e4m3 DoubleRow matmuls (2 contraction
# rows per PE cell, ~1.8x).  Weights are pre-boosted by WBOOST so their
# entries sit in e4m3's normal range; the boost is divided back out in the
# PSUM->SBUF activation copy (scale=).  V / Wo stay bf16: their quantization
# noise would hit the output directly, while q/k noise only perturbs scores.
WBOOST = 64.0
EXP_C0 = 0.99773437
EXP_C1 = 1.0064234
EXP_C2 = 0.5302388
EXP_C3 = 0.16039258

# fused divide out = in0/in1 via the BITWISE_NOT exponent-flip seed:
# 1/b = ~b · g(t) with t = b·~b ∈ [-4.5, -4]; g = deg-2 rel-minimax of 1/t
# (max rel err 5.1e-5).  out = (in0·~b)·(c0 + t·(c1 + t·c2)) — 8 ALU ops,
# exactly the DVE slice budget, one instruction.
DIV_C0 = -0.7071066070499812
DIV_C1 = -0.16652203635848312
DIV_C2 = -0.013060548941608689

_DVE_OPS = {}
DEBUG_DUMP = False


def _register_dve_exp_ops():
    """Register the custom DVE micro-ops (exp path + fused divide).
    Idempotent; appends to concourse's module-level op registry."""
    if _DVE_OPS:
        return _DVE_OPS
    from concourse.dve_spec import (
        Spec, Src0, Src1, C0, C1, C2, C3, Bin, AluOp, sq, lower, _has_src1,
        _spill_c3_to_src1,
    )
    from concourse.dve_uop import DveOpSpec
    from concourse.dve_ops import (
        DveOp,
        OPS,
        CUSTOM_DVE_SPECS,
        _SUB_OPCODE_FOR_NAME,
        _CUSTOM_DVE_ROW_BASE,
    )

    u = Src0

    def _ref_exp4p(in0, in1, s0, s1, imm2):
        p = s0 + in0 * (s1 + in0 * (imm2 + in0 * in1))
        return (p * p) ** 2

    _notb = Bin(AluOp.BITWISE_NOT, Src1, Src1)
    _t = Src1 * _notb

    def _ref_divab(in0, in1, s0, s1, imm2):
        import numpy as np

        notb = (~in1.view(np.int32)).view(np.float32)
        t = in1 * notb
        return (in0 * notb) * (s0 + t * (s1 + t * imm2))

    specs = {
        "ANT_EXP4P": Spec(
            body=_spill_c3_to_src1(sq(sq(C0 + u * (C1 + u * (C2 + u * C3))))),
            reference=_ref_exp4p,
        ),
        "ANT_DIVAB": Spec(
            body=(Src0 * _notb) * (C0 + _t * (C1 + _t * C2)),
            reference=_ref_divab,
        ),
    }
    for name, sp in specs.items():
        if name not in _SUB_OPCODE_FOR_NAME:
            row = _CUSTOM_DVE_ROW_BASE + len(OPS)
            op = DveOp(name, sp, subdim=False, uops_sha={})
            _SUB_OPCODE_FOR_NAME[name] = row
            OPS.append(op)
            CUSTOM_DVE_SPECS[name] = sp
            for ver in ("v3", "v4"):
                s = DveOpSpec(
                    name=name, opcode=row, uops=lower(sp, ver=ver),
                    rd1_en=_has_src1(sp),
                )
                op.uops_sha[ver] = s.sha(ver)
        else:
            op = next(o for o in OPS if o.name == name)
        _DVE_OPS[name] = op
    return _DVE_OPS


def build_attention_nc(seq: int = S) -> bass.Bass:
    KB = seq // P  # key blocks
    DKB = D // P  # 8 contraction blocks for projections
    QH = min(512, seq)  # query stripe processed per attention pass
    NQH = seq // QH
    QC = min(512, QH)  # matmul moving-operand chunk
    NQC = seq // QC  # chunks per full seq
    DT = D // P
    XC = 512  # x DMA column-chunk width
    NXC = seq // XC

    ops = _register_dve_exp_ops()
    exp4p = ops["ANT_EXP4P"]
    divab = ops["ANT_DIVAB"]

    nc = bacc_mod.Bacc("TRN2", num_devices=NCORES)
    xt_d = nc.declare_dram_parameter("xt", [D, seq], BF16, isOutput=False)
    wqt_d = nc.declare_dram_parameter("wqt", [D, DG], BF16, isOutput=False)
    wkt_d = nc.declare_dram_parameter("wkt", [D, DG], BF16, isOutput=False)
    wvt_d = nc.declare_dram_parameter("wvt", [D, DG], BF16, isOutput=False)
    wot_d = nc.declare_dram_parameter("wot", [DG, D], BF16, isOutput=False)
    bq_d = nc.declare_dram_parameter("bqs", [P, FT], F32, isOutput=False)
    out_d = nc.declare_dram_parameter("out", [D, seq], BF16, isOutput=True)
    dbg_d = None
    if DEBUG_DUMP:
        dbg_d = {
            name: nc.declare_dram_parameter(f"dbg_{name}", [P, FT * seq], BF16,
                                            isOutput=True)
            for name in ("qt", "kt", "onorm")
        }
        dbg_d["v"] = nc.declare_dram_parameter("dbg_v", [P, seq // P * DG],
                                               BF16, isOutput=True)

    with tile.TileContext(nc) as tc:
        with tc.tile_pool(name="persist", bufs=1) as persist:
            bq_sb = persist.tile([P, FT], F32, name="bq_sb")
            nc.sync.dma_start(bq_sb, bq_d[:, :])
            c3_sb = persist.tile([P, 1], F32, name="c3_sb")
            nc.vector.memset(c3_sb, EXP_C3)

            qt_sb = [persist.tile([P, seq], BF16, name=f"qt{i}") for i in range(FT)]
            kt_sb = [persist.tile([P, seq], BF16, name=f"kt{i}") for i in range(FT)]
            # plain V tiles: [128 keys, 512 features] per key block
            v_sb = [persist.tile([P, DG], BF16, name=f"v{i}") for i in range(KB)]
            ones_sb = persist.tile([P, DK], BF16, name="ones_sb")
            nc.vector.memset(ones_sb, 1.0)
            wot_sb = [persist.tile([P, D], BF16, name=f"wot{i}") for i in range(FT)]
            onorm = [persist.tile([P, seq], BF16, name=f"onorm{i}") for i in range(FT)]

            # ---------------- phase 1: projections ----------------
            with (
                tc.tile_pool(name="xw", bufs=1) as xw_pool,
                tc.tile_pool(name="pps", bufs=4, space="PSUM") as proj_ps,
            ):
                # Slab tiles: each holds all 8 contraction sub-tiles side by
                # side so ONE dma_start loads a whole slab.
                xt_all = xw_pool.tile([P, DKB * seq], BF16, name="xts")
                wq_all = xw_pool.tile([P, DKB * DG], BF16, name="wqts")
                wk_all = xw_pool.tile([P, DKB * DG], BF16, name="wkts")
                wv_all = xw_pool.tile([P, DKB * DG], BF16, name="wvts")
                xt_sb = [xt_all[:, i * seq : (i + 1) * seq] for i in range(DKB)]
                wqt_sb = [wq_all[:, i * DG : (i + 1) * DG] for i in range(DKB)]
                wkt_sb = [wk_all[:, i * DG : (i + 1) * DG] for i in range(DKB)]
                wvt_sb = [wv_all[:, i * DG : (i + 1) * DG] for i in range(DKB)]

                def wdma(dst_all, w_d):
                    return (
                        dst_all.rearrange("p (k g) -> p k g", g=DG),
                        w_d.rearrange("(k p) g -> p k g", p=P),
                    )

                def xsrc(c):
                    csl = slice(c * XC, (c + 1) * XC)
                    return (
                        xt_all.rearrange("p (k s) -> p k s", s=seq)[:, :, csl],
                        xt_d.rearrange("(k p) s -> p k s", p=P)[:, :, csl],
                    )

                xd0, xs0 = xsrc(0)
                nc.sync.dma_start(xd0[:, 0:4, :], xs0[:, 0:4, :])
                nc.sync.dma_start(xd0[:, 4:8, :], xs0[:, 4:8, :])
                wd, ws = wdma(wq_all, wqt_d)
                nc.scalar.dma_start(wd[:, 0:4, :], ws[:, 0:4, :])
                nc.scalar.dma_start(wd[:, 4:8, :], ws[:, 4:8, :])
                nc.scalar.dma_start(*wdma(wk_all, wkt_d))
                for c in range(1, NXC):
                    (nc.sync if c % 2 else nc.scalar).dma_start(*xsrc(c))
                nc.scalar.dma_start(*wdma(wv_all, wvt_d))
                for ft in range(FT):
                    nc.sync.dma_start(wot_sb[ft], wot_d[ft * P : (ft + 1) * P, :])

                # PE warm-up: the HAM clock gate holds the array at 1.2 GHz
                # until ~3.4us of sustained activity.  Dummy matmuls on a
                # memset tile bridge the input-DMA wait so the first real
                # projection matmuls issue at 2.4 GHz.
                wsrc = xw_pool.tile([P, QC], BF16, name="wsrc")
                nc.vector.memset(wsrc, 0.0)
                wps = proj_ps.tile([P, QC], F32, name="wps", tag="proj")
                for _ in range(24):
                    nc.tensor.matmul(
                        wps, lhsT=wsrc[:, 0:P], rhs=wsrc,
                        start=True, stop=True, skip_group_check=True,
                    )

                # QT / KT: features on partitions, queries on free dim.
                # Chunk 0 of QT runs k 0:4 for all fts first: its x half-slab
                # lands ~3us before the second half, so the wait is filled
                # with useful accumulation.
                for c in range(NQC):
                    csl = slice(c * QC, (c + 1) * QC)
                    psq_c0 = []
                    if c == 0:
                        for ft in range(FT):
                            fsl = slice(ft * P, (ft + 1) * P)
                            psq = proj_ps.tile([P, QC], F32, name="psq",
                                               tag="proj")
                            psq_c0.append(psq)
                            for k in range(4):
                                nc.tensor.matmul(
                                    psq,
                                    lhsT=wqt_sb[k][:, fsl],
                                    rhs=xt_sb[k][:, csl],
                                    start=k == 0,
                                    stop=False,
                                )
                    for ft in range(FT):
                        fsl = slice(ft * P, (ft + 1) * P)
                        if c == 0:
                            psq = psq_c0[ft]
                            krange = range(4, DKB)
                        else:
                            psq = proj_ps.tile([P, QC], F32, name="psq",
                                               tag="proj")
                            krange = range(DKB)
                        for k in krange:
                            nc.tensor.matmul(
                                psq,
                                lhsT=wqt_sb[k][:, fsl],
                                rhs=xt_sb[k][:, csl],
                                start=k == 0,
                                stop=k == DKB - 1,
                            )
                        nc.scalar.activation(
                            qt_sb[ft][:, csl], psq, AF.Identity,
                            bias=bq_sb[:, ft : ft + 1],
                        )
                    for ft in range(FT):
                        fsl = slice(ft * P, (ft + 1) * P)
                        psk = proj_ps.tile([P, QC], F32, name="psk", tag="proj")
                        for k in range(DKB):
                            nc.tensor.matmul(
                                psk,
                                lhsT=wkt_sb[k][:, fsl],
                                rhs=xt_sb[k][:, csl],
                                start=k == 0,
                                stop=k == DKB - 1,
                            )
                        nc.vector.tensor_copy(kt_sb[ft][:, csl], psk)

                # V: keys on partitions, features on free dim.  bv is folded
                # into the host-side bias.  Plain contiguous copy (scalar
                # engine; idle here).
                for kb in range(KB):
                    ksl = slice(kb * P, (kb + 1) * P)
                    psv = proj_ps.tile([P, DG], F32, name="psv", tag="proj")
                    for k in range(DKB):
                        nc.tensor.matmul(
                            psv,
                            lhsT=xt_sb[k][:, ksl],
                            rhs=wvt_sb[k],
                            start=k == 0,
                            stop=k == DKB - 1,
                        )
                    nc.scalar.activation(v_sb[kb], psv, AF.Copy)

            # ---------------- phase 2: attention ----------------
            with (
                tc.tile_pool(name="sps", bufs=2, space="PSUM") as s_ps,
                tc.tile_pool(name="pvps", bufs=2, space="PSUM") as pv_ps,
                tc.tile_pool(name="epool", bufs=8) as e_pool,
                tc.tile_pool(name="dpool", bufs=2) as den_pool,
                tc.tile_pool(name="osb", bufs=4) as o_sb_pool,
            ):
                # Ticks t = (qh, pr, kb) linearized; emitted in batches of
                # two so the PE pays one row->col and one col->row mode
                # drain per TWO ticks.  PV/DEN for tick t are emitted at
                # batch containing t+DEPTH.
                DEPTH = 6  # must be EVEN so (t-DEPTH, t-DEPTH+1) aligns
                # with tick-pair boundaries (kb0's start=True first)
                TICKS = NQH * FT * KB

                def tick_coords(t):
                    qh, r = divmod(t, FT * KB)
                    pr, kb = divmod(r, KB)
                    return qh, pr, kb

                epipe = {}  # t -> (pv, den, pr, kb, e0, e1)
                pend = []  # (due_batch, closure) for norm ops
                pvden = {}  # pr-slot -> (pv, den) current accumulators

                def emit_scores(t):
                    qh, pr, kb = tick_coords(t)
                    qsl = slice(qh * QH, (qh + 1) * QH)
                    ksl = slice(kb * P, (kb + 1) * P)
                    s0 = s_ps.tile([P, QH], F32, name="s0", tag="s0")
                    s1 = s_ps.tile([P, QH], F32, name="s1", tag="s1")
                    nc.tensor.matmul(
                        s0,
                        lhsT=kt_sb[pr][0:64, ksl],
                        rhs=qt_sb[pr][0:64, qsl],
                        start=True, stop=True,
                    )
                    nc.tensor.matmul(
                        s1,
                        lhsT=kt_sb[pr][64:128, ksl],
                        rhs=qt_sb[pr][64:128, qsl],
                        start=True, stop=True,
                    )
                    e0 = e_pool.tile([P, QH], BF16, name="e0", tag="e0")
                    e1 = e_pool.tile([P, QH], BF16, name="e1", tag="e1")
                    nc.scalar.activation(e0, s0, AF.Exp, scale=SC)
                    nc.vector._custom_dve(
                        exp4p, out=e1, in0=s1, in1=c3_sb[:, 0:1],
                        s0=EXP_C0, s1=EXP_C1, imm2=EXP_C2,
                    )
                    if kb == 0:
                        pv = pv_ps.tile([P, QH], F32, name="pv", tag="pv")
                        den = pv_ps.tile([P, QH], F32, name="den", tag="den")
                        pvden[t // KB] = (pv, den)
                    pv, den = pvden[t // KB]
                    epipe[t] = (pv, den, pr, kb, e0, e1)

                def emit_pvden(t):
                    pv, den, pr, kb, e0, e1 = epipe.pop(t)
                    vc = slice(pr * P, pr * P + 64)
                    vc1 = slice(pr * P + 64, (pr + 1) * P)
                    nc.tensor.matmul(
                        pv[0:64, :], lhsT=v_sb[kb][:, vc], rhs=e0,
                        start=kb == 0, stop=kb == KB - 1,
                        tile_position=(0, 0),
                    )
                    nc.tensor.matmul(
                        pv[64:128, :], lhsT=v_sb[kb][:, vc1], rhs=e1,
                        start=kb == 0, stop=kb == KB - 1,
                        tile_position=(0, 64),
                    )
                    nc.tensor.matmul(
                        den[0:64, :], lhsT=ones_sb, rhs=e0,
                        start=kb == 0, stop=kb == KB - 1,
                        tile_position=(0, 0),
                    )
                    nc.tensor.matmul(
                        den[64:128, :], lhsT=ones_sb, rhs=e1,
                        start=kb == 0, stop=kb == KB - 1,
                        tile_position=(0, 64),
                    )
                    if kb == KB - 1:
                        qh, pr_, _ = tick_coords(t)
                        qsl = slice(qh * QH, (qh + 1) * QH)

                        # DVE may read only ONE operand from PSUM: the
                        # scalar engine (same act table as Exp, no table
                        # reload) first copies den to SBUF, then ONE custom
                        # DVE fused-divide normalizes both heads at once.
                        state = {}

                        def norm_copy(den=den, state=state):
                            den_sb = den_pool.tile([P, QH], F32,
                                                   name="den_sb", tag="den_sb")
                            nc.scalar.activation(den_sb, den, AF.Copy)
                            state["den_sb"] = den_sb

                        def norm_div(pv=pv, pr_=pr_, qsl=qsl, state=state):
                            nc.vector._custom_dve(
                                divab, out=onorm[pr_][:, qsl], in0=pv,
                                in1=state["den_sb"],
                                s0=DIV_C0, s1=DIV_C1, imm2=DIV_C2,
                            )

                        # PVDEN(t) lands in batch t//2 + DEPTH//2; queue the
                        # ACT den-copy after it so the strict ACT FIFO never
                        # blocks on the still-pending final DEN matmul.
                        pend.append((t // 2 + DEPTH // 2 + 1, norm_copy))
                        pend.append((t // 2 + DEPTH // 2 + 2, norm_div))

                def flush_due(bi):
                    while pend and pend[0][0] <= bi:
                        pend.pop(0)[1]()
                    pend.sort(key=lambda e: e[0])

                for bi in range(TICKS // 2):
                    t0 = 2 * bi
                    emit_scores(t0)
                    emit_scores(t0 + 1)
                    c0 = t0 - DEPTH  # always odd-aligned pair (c0, c0+1)
                    if c0 >= 0:
                        emit_pvden(c0)
                        emit_pvden(c0 + 1)
                    flush_due(bi)
                # drain the pipe
                for t in sorted(epipe.keys()):
                    emit_pvden(t)
                flush_due(1 << 30)
                while pend:
                    pend.pop(0)[1]()

                if DEBUG_DUMP:
                    for i in range(FT):
                        nc.sync.dma_start(
                            dbg_d["qt"][:, i * seq:(i + 1) * seq], qt_sb[i])
                        nc.sync.dma_start(
                            dbg_d["kt"][:, i * seq:(i + 1) * seq], kt_sb[i])
                        nc.sync.dma_start(
                            dbg_d["onorm"][:, i * seq:(i + 1) * seq], onorm[i])
                    for kb in range(KB):
                        nc.sync.dma_start(
                            dbg_d["v"][:, kb * DG:(kb + 1) * DG], v_sb[kb])

                # ------------ phase 3: output projection ------------
                # dt-outer, c-inner: each wot stationary is reused across
                # the 4 query chunks, so its LDWEIGHTS hides behind the
                # previous same-stationary stream (projection-phase pattern).
                # The 4 concurrent pso accumulators draw one tile from each
                # attention PSUM tag ring (8 banks total -> dt groups are
                # double-buffered, no pool-close barrier).
                PSO_TAGS = ("pv", "den", "s0", "s1")
                PSO_POOL = {"pv": pv_ps, "den": pv_ps, "s0": s_ps, "s1": s_ps}
                for dt in range(DT):
                    dsl = slice(dt * P, (dt + 1) * P)
                    pso = [
                        PSO_POOL[PSO_TAGS[c]].tile([P, QC], F32, name="pso",
                                                   tag=PSO_TAGS[c])
                        for c in range(NQC)
                    ]
                    for ft in range(FT):
                        for c in range(NQC):
                            csl = slice(c * QC, (c + 1) * QC)
                            nc.tensor.matmul(
                                pso[c],
                                lhsT=wot_sb[ft][:, dsl],
                                rhs=onorm[ft][:, csl],
                                start=ft == 0,
                                stop=ft == FT - 1,
                            )
                    for c in range(NQC):
                        csl = slice(c * QC, (c + 1) * QC)
                        o_sb = o_sb_pool.tile([P, QC], BF16, name="o_sb",
                                              tag="osb")
                        nc.vector.tensor_copy(o_sb, pso[c])
                        nc.sync.dma_start(out_d[dsl, csl], o_sb)

    return nc


_CACHE: dict = {}


def _get_nc(seq: int = S) -> bass.Bass:
    key = f"nc{seq}"
    if key not in _CACHE:
        nc = build_attention_nc(seq)
        nc.finalize()  # runs Bacc.compile(): reg alloc + wait legalization
        _CACHE[key] = nc
    return _CACHE[key]


def make_in_maps(x, Wq, bq, Wk, Wv, bv, Wo, seq: int = S):
    bf = ml_dtypes.bfloat16
    scale = 1.0 / (SC * math.sqrt(DK))
    x = np.asarray(x, np.float32)
    Wq = np.asarray(Wq, np.float32)
    bq = np.asarray(bq, np.float32)
    Wk = np.asarray(Wk, np.float32)
    Wv = np.asarray(Wv, np.float32)
    bv = np.asarray(bv, np.float32)
    Wo = np.asarray(Wo, np.float32)
    in_maps = []
    for core in range(NCORES):
        b, g = divmod(core, GROUPS)
        gsl = slice(g * DG, (g + 1) * DG)
        in_maps.append(
            {
                "xt": np.ascontiguousarray(x[b, :seq, :].T).astype(bf),
                "wqt": np.ascontiguousarray((Wq[gsl, :] * scale).T).astype(bf),
                "wkt": np.ascontiguousarray(Wk[gsl, :].T).astype(bf),
                "wvt": np.ascontiguousarray(Wv[gsl, :].T).astype(bf),
                "wot": np.ascontiguousarray(Wo[:, gsl].T).astype(bf),
                "bqs": np.ascontiguousarray(
                    (bq[gsl] * scale).astype(np.float32).reshape(FT, P).T
                ),
            }
        )
    return in_maps


def run_device(in_maps, seq: int = S, trace: bool = False):
    nc = _get_nc(seq)
    return run_bass_kernel_spmd(nc, in_maps, list(range(NCORES)), trace=trace)


def kernel(x, Wq, bq, Wk, bk, Wv, bv, Wo, bo):
    in_maps = make_in_maps(x, Wq, bq, Wk, Wv, bv, Wo)
    res = run_device(in_maps).results
    # bv passes through the attention average unchanged (weights sum to 1),
    # so its contribution to the output is exactly Wo @ bv, added here.
    bias = np.asarray(bo, np.float32) + np.asarray(Wo, np.float32) @ np.asarray(
        bv, np.float32
    )
    out = np.empty((B, S, D), np.float32)
    for b in range(B):
        acc = res[2 * b]["out"].astype(np.float32) + res[2 * b + 1]["out"].astype(
            np.float32
        )
        out[b] = acc.T + bias[None, :]
    return out
